# revision 1
# baseline (speedup 1.0000x reference)
"""Trainium2 Bass kernel for nn_CrossEpochAttention (B=8192, S=10, D=1024, H=8).

Strategy: pure data parallel over 8 NeuronCores (1024 batches each). Per core:
  pass 1: QT/KT projections (weights stationary, fp32r), V projection
          (xT stationary), fused block-diagonal attention with additive
          pos-bias mask, softmax without max-subtraction (scores are O(1)),
          normalization folded into P before the PE transpose, OT written to
          a DRAM scratch.
  pass 2: final Wo projection reading OT tiles (fp32r, N=512).
Host side transposes x and the weights so every DMA is contiguous.
"""

import os
import sys

for _p in (
    "/root/.axon_site",
    "/root/.axon_site/_ro/trn_rl_repo",
    "/root/.axon_site/_ro/pypackages",
    "/opt/trn_rl_repo",
):
    if os.path.isdir(_p) and _p not in sys.path:
        sys.path.append(_p)

import numpy as np

import concourse.bacc as bacc
import concourse.mybir as mybir
import concourse.tile as tile
from concourse import bass_utils
from concourse.masks import make_identity

F32 = mybir.dt.float32
F32R = mybir.dt.float32r
BF16 = mybir.dt.bfloat16
AF = mybir.ActivationFunctionType
ADD = mybir.AluOpType.add

# problem constants
B, S, D = 8192, 10, 1024
H, HD = 8, 128
NCORES = 8
B_LOC = B // NCORES            # 1024
TOK = B_LOC * S                # 10240
GSZ = 120                      # 12 batches per attention group
RSZ = 40                       # runt group: 4 batches
T_CHUNK = 480                  # 4 groups per chunk
SCALE = float(HD) ** -0.5
MASK_NEG = -30000.0
KD = D // 128                  # 8 contraction tiles


def _chunks():
    """(t0, T, [(goff, gsz), ...]) per chunk; 28 full chunks + 160-token runt."""
    out = []
    t0 = 0
    while t0 + T_CHUNK <= 85 * GSZ:  # full 120-groups cover 85*120=10200
        n = min(T_CHUNK, 85 * GSZ - t0)
        if n < T_CHUNK:
            break
        out.append((t0, T_CHUNK, [(g * GSZ, GSZ) for g in range(T_CHUNK // GSZ)]))
        t0 += T_CHUNK
    # tail: remaining full groups + the 40-token runt group
    rem_groups = []
    goff = 0
    while t0 + goff < 85 * GSZ:
        rem_groups.append((goff, GSZ))
        goff += GSZ
    rem_groups.append((goff, RSZ))
    out.append((t0, goff + RSZ, rem_groups))
    assert t0 + goff + RSZ == TOK
    return out


def _build():
    nc = bacc.Bacc("TRN2", target_bir_lowering=False, debug=False,
                   num_devices=NCORES)

    xT = nc.dram_tensor("xT", [128, KD, TOK], F32R, kind="ExternalInput")
    wq = nc.dram_tensor("wqT", [128, KD, D], F32R, kind="ExternalInput")
    wk = nc.dram_tensor("wkT", [128, KD, D], F32R, kind="ExternalInput")
    wv = nc.dram_tensor("wvT", [128, KD, D], F32R, kind="ExternalInput")
    wo = nc.dram_tensor("woT", [128, KD, D], BF16, kind="ExternalInput")
    bqs = nc.dram_tensor("bq_s", [128, KD], F32, kind="ExternalInput")
    bk_ = nc.dram_tensor("bk_l", [128, KD], F32, kind="ExternalInput")
    bvb = nc.dram_tensor("bv_b", [128, D], F32, kind="ExternalInput")
    bob = nc.dram_tensor("bo_b", [128, D], F32, kind="ExternalInput")
    mkf = nc.dram_tensor("mask_full", [GSZ, GSZ], F32, kind="ExternalInput")
    mkr = nc.dram_tensor("mask_runt", [RSZ, RSZ], F32, kind="ExternalInput")
    out = nc.dram_tensor("out", [TOK, D], F32, kind="ExternalOutput")

    chunks = _chunks()

    with tile.TileContext(nc) as tc:
        with tc.tile_pool(name="const", bufs=1) as cpool, \
             tc.tile_pool(name="dram", bufs=1, space="DRAM") as dpool:
            ident = cpool.tile([128, 128], F32)
            make_identity(nc, ident[:])
            identb = cpool.tile([128, 128], BF16)
            nc.vector.tensor_copy(identb[:], ident[:])
            mkf_sb = cpool.tile([GSZ, GSZ], F32)
            nc.sync.dma_start(mkf_sb[:], mkf.ap())
            mkr_sb = cpool.tile([RSZ, RSZ], F32)
            nc.sync.dma_start(mkr_sb[:], mkr.ap())
            bqs_sb = cpool.tile([128, KD], F32)
            nc.sync.dma_start(bqs_sb[:], bqs.ap())
            bk_sb = cpool.tile([128, KD], F32)
            nc.sync.dma_start(bk_sb[:], bk_.ap())
            bvb_sb = cpool.tile([128, D], F32)
            nc.sync.dma_start(bvb_sb[:], bvb.ap())
            otdram = dpool.tile([128, KD, TOK], BF16)

            # ---------------- pass 1 ----------------
            with tc.tile_pool(name="w1", bufs=1) as wpool, \
                 tc.tile_pool(name="xt", bufs=2) as xpool, \
                 tc.tile_pool(name="qkv", bufs=1) as qkvpool, \
                 tc.tile_pool(name="ot", bufs=1) as opool, \
                 tc.tile_pool(name="att", bufs=3) as apool, \
                 tc.tile_pool(name="pp", bufs=10) as ppool, \
                 tc.tile_pool(name="sm", bufs=4) as smpool, \
                 tc.tile_pool(name="ps_proj", bufs=2, space="PSUM") as proj_ps, \
                 tc.tile_pool(name="ps_score", bufs=2, space="PSUM") as score_ps, \
                 tc.tile_pool(name="ps_pt", bufs=2, space="PSUM") as pt_ps, \
                 tc.tile_pool(name="ps_ot", bufs=2, space="PSUM") as ot_ps:

                wq_sb = wpool.tile([128, KD, D], F32R, tag="wq")
                nc.sync.dma_start(wq_sb[:], wq.ap())
                wk_sb = wpool.tile([128, KD, D], F32R, tag="wk")
                nc.sync.dma_start(wk_sb[:], wk.ap())
                wv_sb = wpool.tile([128, KD, D], F32R, tag="wv")
                nc.sync.dma_start(wv_sb[:], wv.ap())

                for (t0, T, groups) in chunks:
                    xt_full = xpool.tile([128, KD, T_CHUNK], F32R, tag="xt", name="xt")
                    xt = xt_full[:, :, :T]
                    nc.sync.dma_start(xt, xT.ap()[:, :, t0:t0 + T])

                    TP = T if T >= 256 else 256  # pad fp32r moving dim to >=256
                    qt_full = qkvpool.tile([128, KD, T_CHUNK], F32R, tag="qt", name="qt")
                    qt = qt_full[:, :, :T]
                    kt_full = qkvpool.tile([128, KD, T_CHUNK], F32R, tag="kt", name="kt")
                    kt = kt_full[:, :, :T]
                    for j in range(KD):
                        ps = proj_ps.tile([128, 512], F32, tag="proj", name="proj")[:, :TP]
                        for kk in range(KD):
                            nc.tensor.matmul(
                                ps, wq_sb[:, kk, j * 128:(j + 1) * 128],
                                xt_full[:, kk, :TP],
                                start=(kk == 0), stop=(kk == KD - 1))
                        nc.scalar.activation(qt[:, j, :], ps[:, :T], AF.Identity,
                                             bias=bqs_sb[:, j:j + 1], scale=SCALE)
                        ps = proj_ps.tile([128, 512], F32, tag="proj", name="proj")[:, :TP]
                        for kk in range(KD):
                            nc.tensor.matmul(
                                ps, wk_sb[:, kk, j * 128:(j + 1) * 128],
                                xt_full[:, kk, :TP],
                                start=(kk == 0), stop=(kk == KD - 1))
                        nc.scalar.activation(kt[:, j, :], ps[:, :T], AF.Identity,
                                             bias=bk_sb[:, j:j + 1], scale=1.0)

                    v_sb = qkvpool.tile([128, T_CHUNK // GSZ, D], BF16, tag="v", name="v")
                    for gi, (goff, gsz) in enumerate(groups):
                        for hf in range(2):
                            ps = proj_ps.tile([128, 512], F32, tag="proj", name="proj")[:gsz]
                            for kk in range(KD):
                                nc.tensor.matmul(
                                    ps, xt[:, kk, goff:goff + gsz],
                                    wv_sb[:, kk, hf * 512:(hf + 1) * 512],
                                    start=(kk == 0), stop=(kk == KD - 1))
                            nc.vector.tensor_tensor(
                                v_sb[:gsz, gi, hf * 512:(hf + 1) * 512], ps,
                                bvb_sb[:gsz, hf * 512:(hf + 1) * 512], ADD)

                    ot_sb = opool.tile([128, KD, T_CHUNK], BF16, tag="ot", name="ot")[:, :, :T]
                    for gi, (goff, gsz) in enumerate(groups):
                        msk = mkf_sb if gsz == GSZ else mkr_sb
                        ssum = smpool.tile([GSZ, H], F32, tag="ssum", name="ssum")[:gsz]
                        p_tiles = []
                        for h in range(H):
                            sps = score_ps.tile([GSZ, T_CHUNK], F32, tag="score", name="score")[:gsz, :TP]
                            nc.tensor.matmul(sps, qt_full[:, h, goff:goff + gsz],
                                             kt_full[:, h, :TP],
                                             start=True, stop=True)
                            a_sb = apool.tile([GSZ, GSZ], F32, tag="a", name="a")[:gsz, :gsz]
                            nc.vector.tensor_tensor(
                                a_sb, sps[:, goff:goff + gsz], msk[:gsz, :gsz], ADD)
                            p_sb = ppool.tile([GSZ, GSZ], F32, tag="p", name="p")[:gsz, :gsz]
                            nc.scalar.activation(p_sb, a_sb, AF.Exp,
                                                 accum_out=ssum[:, h:h + 1])
                            p_tiles.append(p_sb)
                        rs = smpool.tile([GSZ, H], F32, tag="rs", name="rs")[:gsz]
                        nc.vector.reciprocal(rs, ssum)
                        for h in range(H):
                            pn = apool.tile([GSZ, GSZ], BF16, tag="pn", name="pn")[:gsz, :gsz]
                            nc.vector.tensor_scalar_mul(pn, p_tiles[h],
                                                        rs[:, h:h + 1])
                            ptp = pt_ps.tile([GSZ, GSZ], BF16, tag="ptp", name="ptp")[:gsz, :gsz]
                            nc.tensor.transpose(ptp, pn, identb[:gsz, :gsz])
                            pt_sb = apool.tile([GSZ, GSZ], BF16, tag="pt", name="pt")[:gsz, :gsz]
                            nc.vector.tensor_copy(pt_sb, ptp)
                            otp = ot_ps.tile([128, GSZ], F32, tag="otp", name="otp")[:, :gsz]
                            nc.tensor.matmul(
                                otp, v_sb[:gsz, gi, h * 128:(h + 1) * 128],
                                pt_sb, start=True, stop=True)
                            nc.vector.tensor_copy(
                                ot_sb[:, h, goff:goff + gsz], otp)
                    nc.sync.dma_start(otdram[:, :, t0:t0 + T], ot_sb)

            # ---------------- pass 2 ----------------
            with tc.tile_pool(name="w2", bufs=1) as w2pool, \
                 tc.tile_pool(name="p2", bufs=2) as p2pool, \
                 tc.tile_pool(name="fin", bufs=3) as fpool, \
                 tc.tile_pool(name="ps_fin", bufs=4, space="PSUM") as fin_ps:
                wo_sb = w2pool.tile([128, KD, D], BF16, tag="wo")
                nc.sync.dma_start(wo_sb[:], wo.ap())
                bob_sb = w2pool.tile([128, D], F32, tag="bo")
                nc.sync.dma_start(bob_sb[:], bob.ap())
                for tt in range(TOK // 512):
                    ot2 = p2pool.tile([128, KD, 512], BF16, tag="ot2", name="ot2")
                    nc.sync.dma_start(ot2[:], otdram[:, :, tt * 512:(tt + 1) * 512])
                    for m in range(4):
                        for hf in range(2):
                            fps = fin_ps.tile([128, 512], F32, tag="fin", name="fin")
                            for kk in range(KD):
                                nc.tensor.matmul(
                                    fps, ot2[:, kk, m * 128:(m + 1) * 128],
                                    wo_sb[:, kk, hf * 512:(hf + 1) * 512],
                                    start=(kk == 0), stop=(kk == KD - 1))
                            f_sb = fpool.tile([128, 512], F32, tag="f", name="f")
                            nc.vector.tensor_tensor(
                                f_sb, fps, bob_sb[:, hf * 512:(hf + 1) * 512], ADD)
                            r0 = tt * 512 + m * 128
                            nc.sync.dma_start(
                                out.ap()[r0:r0 + 128,
                                         hf * 512:(hf + 1) * 512], f_sb)

    nc.compile()
    return nc


_NC = None


def _get_nc():
    global _NC
    if _NC is None:
        _NC = _build()
    return _NC


def _mask(pos_bias, nb):
    """Additive mask [nb*S, nb*S]: pos_bias[q-k+S-1] on the block diagonal,
    MASK_NEG off it."""
    n = nb * S
    q = np.arange(n)
    k = np.arange(n)
    same = (q[:, None] // S) == (k[None, :] // S)
    # reference: rel_idx = rng[None, :] - rng[:, None] + S - 1  (k - q + S - 1)
    rel = (k[None, :] % S) - (q[:, None] % S) + S - 1
    m = np.where(same, pos_bias[rel], np.float32(MASK_NEG))
    return np.ascontiguousarray(m, np.float32)


def _in_maps(x, Wq, bq, Wk, bk, Wv, bv, Wo, bo, pos_bias):
    x = np.asarray(x, np.float32)

    def wlay(w):  # [d_out, d_in] -> [p, kk, d_out] with d_in = kk*128+p
        return np.ascontiguousarray(
            np.asarray(w, np.float32).T.reshape(KD, 128, D).transpose(1, 0, 2))

    def blay(b):  # [d_out] -> [p, j] with d_out = j*128+p
        return np.ascontiguousarray(np.asarray(b, np.float32).reshape(KD, 128).T)

    import ml_dtypes
    common = {
        "wqT": wlay(Wq), "wkT": wlay(Wk), "wvT": wlay(Wv),
        "woT": wlay(Wo).astype(ml_dtypes.bfloat16),
        "bq_s": blay(np.asarray(bq, np.float32) * np.float32(SCALE)),
        "bk_l": blay(bk),
        "bv_b": np.ascontiguousarray(
            np.broadcast_to(np.asarray(bv, np.float32), (128, D))),
        "bo_b": np.ascontiguousarray(
            np.broadcast_to(np.asarray(bo, np.float32), (128, D))),
        "mask_full": _mask(np.asarray(pos_bias, np.float32), GSZ // S),
        "mask_runt": _mask(np.asarray(pos_bias, np.float32), RSZ // S),
    }
    in_maps = []
    for i in range(NCORES):
        xs = x[i * B_LOC:(i + 1) * B_LOC].reshape(TOK, D)
        xTl = np.ascontiguousarray(
            xs.T.reshape(KD, 128, TOK).transpose(1, 0, 2))
        in_maps.append({"xT": xTl, **common})
    return in_maps


def kernel(x, Wq, bq, Wk, bk, Wv, bv, Wo, bo, pos_bias):
    nc = _get_nc()
    in_maps = _in_maps(x, Wq, bq, Wk, bk, Wv, bv, Wo, bo, pos_bias)

    res = bass_utils.run_bass_kernel_spmd(nc, in_maps,
                                          core_ids=list(range(NCORES)))
    return np.concatenate(
        [res.results[i]["out"].reshape(B_LOC, S, D) for i in range(NCORES)],
        axis=0)

